# revision 34
# baseline (speedup 1.0000x reference)
"""Trainium2 Bass kernel for a custom LSTM cell.

reference:
    z = concat([h_tm1, inputs], -1) @ kernel      # [B, 4U]
    i, f, g, o = split(z, 4, -1)
    c = sigmoid(f) * c_tm1 + sigmoid(i) * tanh(g)
    h = sigmoid(o) * tanh(c)
    returns (h, c)

Sharding over 8 NeuronCores: 2-way over batch x 4-way over units
(each gate's block co-located per core).  Per core:
    z_blk = A_half @ W[:, 4 gate slices of 256] in bf16 (full PE rate,
    half the HBM traffic of fp32r), gate math on-chip, outputs
    [1024, 256] h/c blocks in fp32.  Host only slices/casts/concats.

Schedule (per core):
  DMA: two hardware HWDGE rings only (Sync + Scalar; GpSimd is software
       DGE, ~3x slower): at per-k chunks on Sync (k1 rides Scalar right
       behind wk0 for extra early bandwidth), weights on Scalar with
       wk_hi strictly behind wk_lo, ct last.  Chunks sized so arrival
       tracks PE consumption (~1.73us per k-row).
  Warm-up: 10 dummy matmuls on an uninitialized raw SBUF tile, no data
       deps: the PE burns its HAM K=4/8 half-rate window (~4.4us from
       first activity) while the first input chunks are still in flight.
  phase 1: all m, i|f columns, k-outer round-robin over 8 PSUM banks;
       each closes with Sigmoid -> sig_if (bf16), freeing its bank.
  phase 2: m0..m5, then m7's g-half matmuls, then m6, then m7's o-half.
       m7's g-chain (tanh g -> i*g -> +fc -> tanh c -> c-half DMA out)
       completes under m6's matmuls.  m7's o-gate runs as two 128-col
       PSUM groups so the first half's sig -> mul -> DMA proceeds under
       the second half's matmuls (128-col matmul pitch is ~56ns, so the
       extra group is nearly PE-free); after the final matmul only
       sig(o_b) -> mul -> 32KB DMA remain, with the two h-half pieces on
       the Sync and Scalar rings so their completions overlap.

Fixed costs measured from the NTFF traces (not removable from kernel
code): ~3us runtime preamble (per-engine register/ACT-table loads +
barriers), ~1.2us sequencer fetch before the first tile instruction,
~2.6us HWDGE issue->completion latency on the first chunks, and ~8us
runtime-injected teardown (each engine serially resets its ~50-entry
partition of the 256-semaphore file at ~115ns each, plus final
barrier) -- the teardown instructions are injected at NEFF load, they
are not in the walrus-generated engine binaries.
"""

import sys

sys.path.insert(0, "/opt/trn_rl_repo")

import ml_dtypes
import numpy as np

BF16 = ml_dtypes.bfloat16

BATCH, INPUT_DIM, UNITS = 2048, 512, 1024
K = UNITS + INPUT_DIM  # contraction dim, 1536
R, C = 2, 4  # batch halves x unit quarters
BR = BATCH // R  # 1024 batch rows per core
UC = UNITS // C  # 256 units per core
KS = K // 128  # 12 k-subtiles
M = BR // 128  # 8 batch sub-chunks per core

_CACHE = {}


def _build_nc():
    import concourse.tile as tile
    from concourse import bacc, mybir

    f32 = mybir.dt.float32
    bf16 = mybir.dt.bfloat16
    Sig = mybir.ActivationFunctionType.Sigmoid
    Tanh = mybir.ActivationFunctionType.Tanh

    nc = bacc.Bacc("TRN2")
    at_in = nc.declare_dram_parameter("at", [K, BR], bf16, isOutput=False)
    wklo_in = nc.declare_dram_parameter("wk_lo", [K, 512], bf16, isOutput=False)
    wkhi_in = nc.declare_dram_parameter("wk_hi", [K, 512], bf16, isOutput=False)
    ct_in = nc.declare_dram_parameter("ct", [BR, UC], bf16, isOutput=False)
    ch_out = nc.declare_dram_parameter("ch_out", [BR, 512], bf16, isOutput=True)

    # Raw (non-pool) scratch for warm-up matmuls: contents irrelevant,
    # no writer -> no dependencies -> earliest possible PE start.
    dummy = nc.alloc_sbuf_tensor("warm", [128, 512], bf16)

    with tile.TileContext(nc) as tc:
        with (
            tc.tile_pool(name="sb", bufs=1) as sb,
            tc.tile_pool(name="psum", bufs=8, space="PSUM") as psum,
        ):
            at = sb.tile([128, KS, BR], bf16)
            wk_lo = sb.tile([128, KS, 512], bf16)  # i|f columns
            wk_hi = sb.tile([128, KS, 512], bf16)  # g|o columns
            ct = sb.tile([128, M, UC], bf16)
            sig_if = sb.tile([128, M, 512], bf16)
            fc_all = sb.tile([128, M, UC], bf16)

            at_r = at_in[:].rearrange("(ko p) n -> p ko n", p=128)
            wklo_r = wklo_in[:].rearrange("(ko p) n -> p ko n", p=128)
            wkhi_r = wkhi_in[:].rearrange("(ko p) n -> p ko n", p=128)
            ct_r = ct_in[:].rearrange("(m p) u -> p m u", p=128)

            # at on the Sync HWDGE ring, consumption-ordered.  Per-k chunks:
            # the 256KB transfer time dominates the ~0.65us ring cost, and
            # fine granularity turns a slow-core data lag into many sub-1us
            # PE waits (HAM-safe) instead of one >3us stall (HAM drop).
            # (GpSimd's ring is SOFTWARE DGE -- ~3x slower; never use it
            # for bulk data.)
            nc.sync.dma_start(at[:, 0:1, 0:512], at_r[:, 0:1, 0:512])
            nc.sync.dma_start(at[:, 0:1, 512:1024], at_r[:, 0:1, 512:1024])
            # k2/k3 in half-row pieces: the PE exits the HAM window at
            # full rate right as it reaches k2, and catches the ring
            # there -- halving the chunk halves the stall quantum.
            for j in (2, 3):
                nc.sync.dma_start(at[:, j : j + 1, 0:512], at_r[:, j : j + 1, 0:512])
                nc.sync.dma_start(
                    at[:, j : j + 1, 512:1024], at_r[:, j : j + 1, 512:1024]
                )
            for j in range(4, KS):
                nc.sync.dma_start(at[:, j : j + 1, :], at_r[:, j : j + 1, :])
            # weights on the Scalar HWDGE ring; wk_hi strictly behind wk_lo.
            # at-k1 rides the scalar ring right behind wk0: the sync ring
            # alone falls ~2us behind PE consumption around k1-k2 (both
            # rings are cold and every core is pulling at once).
            nc.scalar.dma_start(wk_lo[:, 0:1, :], wklo_r[:, 0:1, :])
            nc.scalar.dma_start(at[:, 1:2, :], at_r[:, 1:2, :])
            for ks in (
                slice(1, 2),
                slice(2, 4),
                slice(4, 6),
                slice(6, 8),
                slice(8, 10),
                slice(10, KS),
            ):
                nc.scalar.dma_start(wk_lo[:, ks, :], wklo_r[:, ks, :])
            for ks in (slice(0, 4), slice(4, 8), slice(8, KS)):
                nc.scalar.dma_start(wk_hi[:, ks, :], wkhi_r[:, ks, :])
            # ct last on the scalar ring FIFO: it is not consumed until the
            # fc multiply at phase-1 end (~35us), so keep its 512KB out of
            # the contended early window that gates the k-row cadence.
            nc.scalar.dma_start(ct[:, :, :], ct_r[:, :, :])

            # phase 1: all m, i|f columns, k-outer round-robin over 8 banks.
            plo = [
                psum.tile([128, 512], f32, tag="ps", name=f"plo{m}") for m in range(M)
            ]
            # 10 dummies cover the typical arrival of the first at/wk
            # chunks (~9.5-10.5us with the split front) so the real stream
            # starts clean, burning the HAM K=4/8 half-rate window (~4.4us
            # from first PE activity) on useless work while data is still
            # in flight.
            for _ in range(10):
                nc.tensor.matmul(
                    plo[0][:],
                    dummy[:, 0:128],
                    dummy[:],
                    start=True,
                    stop=True,
                    skip_group_check=True,
                )
            for k in range(KS):
                for m in range(M):
                    nc.tensor.matmul(
                        plo[m][:],
                        at[:, k, m * 128 : (m + 1) * 128],
                        wk_lo[:, k, :],
                        start=(k == 0),
                        stop=(k == KS - 1),
                    )
            for m in range(M):
                nc.scalar.activation(sig_if[:, m, :], plo[m][:], Sig)
            # f * c_tm1 off the epilogue critical path (DVE is idle here)
            for m in range(M):
                nc.vector.tensor_mul(
                    fc_all[:, m, :], sig_if[:, m, UC : 2 * UC], ct[:, m, :]
                )

            # phase 2: per-m serial g|o accumulation + epilogue.
            # Order: m0..m5, then m7's g-half, then m6, then m7's o-half.
            # m7's g-chain (tanh g -> i*g -> +fc -> tanh c -> c-half DMA)
            # completes UNDER m6's matmuls, so after the final o matmuls
            # the only remaining chain is sig(o) -> o*tanh(c) -> h DMA.
            def gate_epilogue(m, phi_g, phi_o, och):
                ms = slice(m * 128, (m + 1) * 128)
                tg = sb.tile([128, UC], bf16, tag="tg", bufs=3)
                nc.scalar.activation(tg[:], phi_g, Tanh)
                ig = sb.tile([128, UC], bf16, tag="ig", bufs=3)
                nc.vector.tensor_mul(ig[:], sig_if[:, m, 0:UC], tg[:])
                nc.vector.tensor_add(och[:, 0:UC], fc_all[:, m, :], ig[:])
                th = sb.tile([128, UC], bf16, tag="th", bufs=3)
                nc.scalar.activation(th[:], och[:, 0:UC], Tanh)
                so = sb.tile([128, UC], bf16, tag="so", bufs=3)
                nc.scalar.activation(so[:], phi_o, Sig)
                nc.vector.tensor_mul(och[:, UC : 2 * UC], so[:], th[:])
                nc.sync.dma_start(ch_out[ms, :], och[:])

            for m in list(range(6)):
                ms = slice(m * 128, (m + 1) * 128)
                # bufs=4: with 3, the late-m adds wait on output-DMA
                # completion (buffer recycle) right in the final chain.
                och = sb.tile([128, 512], bf16, tag="och", bufs=4)
                phi = psum.tile([128, 512], f32, tag="ps", name=f"phi{m}")
                for k in range(KS):
                    nc.tensor.matmul(
                        phi[:],
                        at[:, k, ms],
                        wk_hi[:, k, :],
                        start=(k == 0),
                        stop=(k == KS - 1),
                    )
                gate_epilogue(m, phi[:, 0:UC], phi[:, UC : 2 * UC], och)

            m7s = slice(7 * 128, 8 * 128)
            och7 = sb.tile([128, 512], bf16, tag="och", bufs=4)
            pg = psum.tile([128, UC], f32, tag="ps", name="pg")
            for k in range(KS):
                nc.tensor.matmul(
                    pg[:],
                    at[:, k, m7s],
                    wk_hi[:, k, 0:UC],
                    start=(k == 0),
                    stop=(k == KS - 1),
                )
            tg7 = sb.tile([128, UC], bf16, tag="tg", bufs=3)
            nc.scalar.activation(tg7[:], pg[:], Tanh)
            ig7 = sb.tile([128, UC], bf16, tag="ig", bufs=3)
            nc.vector.tensor_mul(ig7[:], sig_if[:, 7, 0:UC], tg7[:])
            nc.vector.tensor_add(och7[:, 0:UC], fc_all[:, 7, :], ig7[:])
            th7 = sb.tile([128, UC], bf16, tag="th", bufs=3)
            nc.scalar.activation(th7[:], och7[:, 0:UC], Tanh)
            # c-half leaves early, off the end-of-kernel chain.
            nc.sync.dma_start(ch_out[m7s, 0:UC], och7[:, 0:UC])

            m = 6
            ms = slice(m * 128, (m + 1) * 128)
            och = sb.tile([128, 512], bf16, tag="och", bufs=4)
            phi = psum.tile([128, 512], f32, tag="ps", name="phi6")
            for k in range(KS):
                nc.tensor.matmul(
                    phi[:],
                    at[:, k, ms],
                    wk_hi[:, k, :],
                    start=(k == 0),
                    stop=(k == KS - 1),
                )
            gate_epilogue(6, phi[:, 0:UC], phi[:, UC : 2 * UC], och)

            # m7's o-gate in two 128-col PSUM groups: the first half's
            # sig -> mul -> DMA runs under the second half's matmuls
            # (128-col matmul pitch scales with columns, so the extra
            # group costs almost no PE time), and the very last DMA is
            # only 32KB.  First-half h on the Sync ring, second on
            # Scalar, so the final completions overlap.
            po_a = psum.tile([128, 128], f32, tag="ps", name="po_a")
            for k in range(KS):
                nc.tensor.matmul(
                    po_a[:],
                    at[:, k, m7s],
                    wk_hi[:, k, UC : UC + 128],
                    start=(k == 0),
                    stop=(k == KS - 1),
                )
            so_a = sb.tile([128, 128], bf16, tag="so", bufs=3)
            nc.scalar.activation(so_a[:], po_a[:], Sig)
            nc.vector.tensor_mul(och7[:, UC : UC + 128], so_a[:], th7[:, 0:128])
            nc.sync.dma_start(ch_out[m7s, UC : UC + 128], och7[:, UC : UC + 128])

            po_b = psum.tile([128, 128], f32, tag="ps", name="po_b")
            for k in range(KS):
                nc.tensor.matmul(
                    po_b[:],
                    at[:, k, m7s],
                    wk_hi[:, k, UC + 128 : 2 * UC],
                    start=(k == 0),
                    stop=(k == KS - 1),
                )
            so_b = sb.tile([128, 128], bf16, tag="so", bufs=3)
            nc.scalar.activation(so_b[:], po_b[:], Sig)
            nc.vector.tensor_mul(
                och7[:, UC + 128 : 2 * UC], so_b[:], th7[:, 128:256]
            )
            nc.scalar.dma_start(
                ch_out[m7s, UC + 128 : 2 * UC], och7[:, UC + 128 : 2 * UC]
            )

    # Strip the three unused const Memsets (f32-1.0 / bf16-1.0 / u8-127,
    # emitted unconditionally by Bass.__init__) from 'main': they run
    # serially on the Pool engine right before the main barrier, and Pool
    # is the last barrier arrival, so each delays every engine's kernel
    # entry by ~100ns.  const-f32-0.0 stays (activation bias reads it).
    main_bb = next(b for f in nc.m.functions for b in f.blocks if b.name == "main")
    _drop = ("const-float32-1.0", "const-bfloat16-1.0", "const-uint8-127")

    def _dead(i):
        c = i.concise()
        if "Memset" in c and any(d in c for d in _drop):
            return True
        # Main's all-engine barrier (Drain + gather/release EventSemaphore
        # rounds) only protected the const Memsets: the runtime preamble
        # already ends with its own all-engine barrier, the tile block's
        # cross-engine sync is absolute semaphore counts (skew-tolerant),
        # and the surviving f32-0.0 Memset's first reader is ~23us later.
        # Dropping the full round keeps the gather/release sems balanced.
        return "Drain" in c or "EventSemaphore" in c

    main_bb.instructions = [i for i in main_bb.instructions if not _dead(i)]

    nc.compile()
    return nc


def get_nc():
    if "nc" not in _CACHE:
        _CACHE["nc"] = _build_nc()
    return _CACHE["nc"]


def make_in_maps(inputs, h_tm1, c_tm1, kernel):
    x = np.asarray(inputs, dtype=np.float32)
    h = np.asarray(h_tm1, dtype=np.float32)
    c = np.asarray(c_tm1, dtype=np.float32).astype(BF16)
    w = np.asarray(kernel, dtype=np.float32).astype(BF16)
    at_full = np.ascontiguousarray(
        np.concatenate([h, x], axis=1).T.astype(BF16)
    )  # [K, B] bf16
    in_maps = []
    for core in range(R * C):
        r, ci = divmod(core, C)
        at_np = np.ascontiguousarray(at_full[:, r * BR : (r + 1) * BR])
        gates = [
            w[:, g * UNITS + ci * UC : g * UNITS + (ci + 1) * UC] for g in range(4)
        ]
        wklo_np = np.ascontiguousarray(np.concatenate(gates[0:2], axis=1))
        wkhi_np = np.ascontiguousarray(np.concatenate(gates[2:4], axis=1))
        ct_np = np.ascontiguousarray(c[r * BR : (r + 1) * BR, ci * UC : (ci + 1) * UC])
        in_maps.append(
            {"at": at_np, "wk_lo": wklo_np, "wk_hi": wkhi_np, "ct": ct_np}
        )
    return in_maps


def assemble(results):
    h_new = np.empty((BATCH, UNITS), dtype=np.float32)
    c_new = np.empty((BATCH, UNITS), dtype=np.float32)
    for core in range(R * C):
        r, ci = divmod(core, C)
        ch = results[core]["ch_out"].astype(np.float32)
        c_new[r * BR : (r + 1) * BR, ci * UC : (ci + 1) * UC] = ch[:, 0:UC]
        h_new[r * BR : (r + 1) * BR, ci * UC : (ci + 1) * UC] = ch[:, UC : 2 * UC]
    return h_new, c_new


def kernel(inputs, h_tm1, c_tm1, kernel):
    from concourse.bass_utils import run_bass_kernel_spmd

    nc = get_nc()
    in_maps = make_in_maps(inputs, h_tm1, c_tm1, kernel)
    res = run_bass_kernel_spmd(nc, in_maps, list(range(R * C)), trace=False)
    return assemble(res.results)

